# revision 2
# baseline (speedup 1.0000x reference)
"""ConvLSTM3D Trainium2 kernel, v4: fp8 DoubleRow hybrid, pair-granular.

Same math/layout as v3 (33x33 planes, 6 fp8 DR pairs + 3 bf16 HC slots
per output half, x32 weight prescale) but restructured for PE feed:

- PSUM tiles are seg-PAIRS [128,1024] (2 banks); each seg's 9 matmuls
  target one half. bufs=2 -> all 8 banks, 2 pairs in flight.
- Gate math runs once per PAIR on [*,1024] operands (halves the
  per-op overhead that starved the PE in v3). el-compute (PSUM reads,
  C2 update) is emitted right after its own pair; only the H-state
  dup DMAs are deferred 2 pairs to respect the WAR window (+-1122
  element read reach of the conv).
- All DMAs issue on the sync queue; engines: DVE 6 ops, scalar 4 acts
  + fp8 cast, gpsimd 3 muls per pair.
- X refresh split into 4 pieces interleaved into the previous
  timestep's pair loop (kills the 30us t-boundary PE bubble).
- Full-tile memsets replaced by edge-range memsets; mask is a host
  input loaded once.
"""
import math
import numpy as np
import ml_dtypes
from contextlib import ExitStack

import concourse.bacc as bacc
import concourse.bass as bass
import concourse.tile as tile
import concourse.mybir as mybir
from concourse.ap import AP

f32 = mybir.dt.float32
bf16 = mybir.dt.bfloat16
fp8 = mybir.dt.float8e4
E4 = ml_dtypes.float8_e4m3
BF = ml_dtypes.bfloat16

B, C_IN, T, D, HS, WS = 8, 4, 8, 16, 32, 32
C_OUT = 64
PLR = 33
PL = PLR * PLR
SEG = 512
PAIR = 2 * SEG
WSCALE = 32.0
NPIECE = 4


def geom(d_depth):
    vp = (d_depth + 2) * PL
    pos0 = PL + PLR + 1
    last = d_depth * PL + 32 * PLR + 32
    nseg = math.ceil((last + 1 - pos0) / SEG)
    if nseg % 2:
        nseg += 1                                # pair-granular loop
    vpa = (max(vp + 64, pos0 + nseg * SEG + PL + PLR + 2) + 15) // 16 * 16
    return vp, pos0, nseg, vpa


def d_ha(kd, kh):
    return kd * PL + kh * PLR - 1


def d_hb(kd):
    return kd * PL - PLR + 1


def d_hc(kd):
    return kd * PL + PLR + 1


def pair_offsets(vpa):
    ha = [d_ha(kd, kh) for kd in (-1, 0, 1) for kh in (-1, 0, 1)]
    hb = [vpa + d_hb(kd) for kd in (-1, 0, 1)]
    slots = ha + hb
    return [(slots[2 * k], slots[2 * k + 1]) for k in range(6)]


def build_nc(t_steps=T, d_depth=D):
    vp, pos0, nseg, vpa = geom(d_depth)
    segtot = nseg * SEG
    npair = nseg // 2
    pairs = pair_offsets(vpa)

    nc = bacc.Bacc("TRN2", target_bir_lowering=False, debug=False)

    xpad_h = nc.dram_tensor("xpad", [C_IN, t_steps, vp], bf16, kind="ExternalInput")
    ww8_h = nc.dram_tensor("ww8", [128, 12 * 256], fp8, kind="ExternalInput")
    ww16_h = nc.dram_tensor("ww16", [128, 3 * 256], bf16, kind="ExternalInput")
    bias_h = nc.dram_tensor("bias", [128, 4], f32, kind="ExternalInput")
    pp_h = nc.dram_tensor("pp", [128, segtot], fp8, kind="ExternalInput")
    ppom_h = nc.dram_tensor("ppom", [128, segtot], fp8, kind="ExternalInput")
    y_h = nc.dram_tensor("y", [64, t_steps * segtot], bf16, kind="ExternalOutput")

    with tile.TileContext(nc) as tc, ExitStack() as ctx:
        const = ctx.enter_context(tc.tile_pool(name="const", bufs=1))
        sc = ctx.enter_context(tc.tile_pool(name="sc", bufs=2))
        mosc = ctx.enter_context(tc.tile_pool(name="mosc", bufs=2))
        hqp = ctx.enter_context(tc.tile_pool(name="hqp", bufs=2))
        psum = ctx.enter_context(tc.tile_pool(name="psum", bufs=2, space="PSUM"))

        HH8 = const.tile([128, 2 * vpa], fp8)
        HC16 = const.tile([128, vpa], bf16)
        C2 = const.tile([128, segtot], bf16)
        PP = const.tile([128, segtot], fp8)
        PPM = const.tile([128, segtot], fp8)   # rows 0-63 mask, 64-127 ppo
        WW8 = const.tile([128, 12 * 256], fp8)
        WW16 = const.tile([128, 3 * 256], bf16)
        BIAS = const.tile([128, 4], f32)

        # full zero-init of H regions (pads are load-bearing; uninit SBUF
        # may encode NaN and NaN*0 poisons PSUM). Split across engines.
        nc.vector.memset(HH8[:, 0:vpa], 0.0)
        nc.gpsimd.memset(HH8[:, vpa:2 * vpa], 0.0)
        nc.gpsimd.memset(HC16[:], 0.0)

        nc.sync.dma_start(WW8[:], ww8_h[:])
        nc.sync.dma_start(WW16[:], ww16_h[:])
        nc.sync.dma_start(BIAS[:], bias_h[:])
        nc.sync.dma_start(PP[:], pp_h[:])
        nc.sync.dma_start(PPM[:], ppom_h[:])

        ww8_3 = WW8[:].rearrange("p (s m) -> p s m", m=256)
        ww16_3 = WW16[:].rearrange("p (s m) -> p s m", m=256)
        hh8f = HH8[:]
        hc16f = HC16[:]

        b_if = BIAS[:, 0:1]
        b_c = BIAS[0:64, 1:2]
        b_o = BIAS[0:64, 2:3]
        b_zero = BIAS[0:64, 3:4]
        inv = 1.0 / WSCALE

        def _ap3(base_ap, off0, stride, n2, nlast):
            return AP(base_ap.tensor, base_ap.offset + off0,
                      [list(base_ap.ap[0]), [stride, n2], [1, nlast]])

        def emit_x_piece(t, a, b2):
            n = b2 - a
            src_base = xpad_h[:]
            for ka in range(3):
                hb = HC16[64 + 12 * ka:64 + 12 * ka + 12, :]
                dst = AP(hb.tensor, hb.offset + a, [list(hb.ap[0]), [1, n]])
                src = AP(src_base.tensor,
                         src_base.offset + t * vp + a - 68 + PLR * ka,
                         [[1, 3], [t_steps * vp, 4], [1, n]])
                nc.sync.dma_start(dst, src)

        xlo = 68
        plen = math.ceil((vp - xlo) / NPIECE)
        xpieces = [(xlo + k * plen, min(vp, xlo + (k + 1) * plen))
                   for k in range(NPIECE)]
        # emit piece after the last PAIR whose segs read its cols
        xpiece_after = [min(npair - 1, ((b2 - 1 - 68) // SEG) // 2)
                        for (a, b2) in xpieces]

        def emit_mm(t, g):
            """Both segs (2g, 2g+1) of pair g -> p0/p1 [128,1024]."""
            p0 = psum.tile([128, PAIR], f32, tag="p0")
            p1 = psum.tile([128, PAIR], f32, tag="p1")
            for h in range(2):
                s = 2 * g + h
                base = pos0 + SEG * s
                hsl = slice(h * SEG, h * SEG + SEG)
                for mt, ptile in ((0, p0), (1, p1)):
                    msl = slice(mt * 128, (mt + 1) * 128)
                    first = True
                    if t > 0:
                        for k, (oa, ob) in enumerate(pairs):
                            nc.tensor.matmul(
                                ptile[:, hsl],
                                ww8_3[:, 2 * k:2 * k + 2, msl],
                                _ap3(hh8f, base + oa, ob - oa, 2, SEG),
                                start=first, stop=False, skip_group_check=True,
                                perf_mode=mybir.MatmulPerfMode.DoubleRow,
                            )
                            first = False
                    for j, kd in enumerate((-1, 0, 1)):
                        off = base + d_hc(kd)
                        nc.tensor.matmul(
                            ptile[:, hsl], ww16_3[:, j, msl],
                            hc16f[:, off:off + SEG],
                            start=first, stop=(j == 2), skip_group_check=True,
                        )
                        first = False
            return p0, p1

        def emit_elc(t, g, p0, p1):
            """Pair-granular gate math; frees p0/p1; writes C2 + hf/hq."""
            i0 = PAIR * g
            isl = slice(i0, i0 + PAIR)
            G = sc.tile([128, PAIR], bf16, tag="g")
            if t > 0:
                mfg = sc.tile([128, PAIR], bf16, tag="mf")
                nc.vector.tensor_mul(mfg[:], PP[:, isl], C2[:, isl])
                nc.vector.tensor_add(mfg[:], mfg[:], p0[:])
                nc.scalar.activation(G[:], mfg[:],
                                     mybir.ActivationFunctionType.Sigmoid,
                                     bias=b_if, scale=inv)
            else:
                nc.scalar.activation(G[:], p0[:],
                                     mybir.ActivationFunctionType.Sigmoid,
                                     bias=b_if, scale=inv)
            tc_t = mosc.tile([64, PAIR], bf16, tag="tt")
            nc.scalar.activation(tc_t[:], p1[0:64, :],
                                 mybir.ActivationFunctionType.Tanh,
                                 bias=b_c, scale=inv)
            tcm = mosc.tile([64, PAIR], bf16, tag="tcm")
            nc.vector.tensor_mul(tcm[:], tc_t[:], PPM[0:64, isl])
            if t > 0:
                vw = mosc.tile([64, PAIR], bf16, tag="vw")
                nc.vector.tensor_mul(vw[:], G[0:64, :], tcm[:])
                fC = mosc.tile([64, PAIR], bf16, tag="fc")
                nc.gpsimd.tensor_mul(fC[:], G[64:128, :], C2[64:128, isl])
                nc.vector.tensor_add(C2[0:64, isl], vw[:], fC[:])
            else:
                nc.vector.tensor_mul(C2[0:64, isl], G[0:64, :], tcm[:])
            nc.sync.dma_start(C2[64:128, isl], C2[0:64, isl])
            th = mosc.tile([64, PAIR], bf16, tag="th")
            nc.scalar.activation(th[:], C2[0:64, isl],
                                 mybir.ActivationFunctionType.Tanh,
                                 bias=b_zero)
            mo = mosc.tile([64, PAIR], bf16, tag="mo")
            nc.gpsimd.tensor_mul(mo[:], PPM[64:128, isl], C2[64:128, isl])
            mo2 = mosc.tile([64, PAIR], bf16, tag="mo2")
            nc.vector.tensor_add(mo2[:], mo[:], p1[64:128, :])
            og = mosc.tile([64, PAIR], bf16, tag="og")
            nc.scalar.activation(og[:], mo2[:],
                                 mybir.ActivationFunctionType.Sigmoid,
                                 bias=b_o, scale=inv)
            hf = hqp.tile([64, PAIR], bf16, tag="hf")
            nc.gpsimd.tensor_mul(hf[:], og[:], th[:])   # masked via th
            hq = hqp.tile([64, PAIR], fp8, tag="hq")
            nc.scalar.activation(hq[:], hf[:],
                                 mybir.ActivationFunctionType.Copy)
            return hf, hq

        def emit_dups(t, g, hf, hq):
            i0 = PAIR * g
            g0 = pos0 + i0
            hqf = hq[:]
            hff = hf[:]
            src = AP(hqf.tensor, hqf.offset, [list(hqf.ap[0]), [0, 2], [1, PAIR]])
            lo_dst = AP(hh8f.tensor, hh8f.offset + g0,
                        [[2 * vpa, 64], [vpa, 2], [1, PAIR]])
            nc.sync.dma_start(lo_dst, src)
            up_dst = AP(hh8f.tensor, hh8f.offset + 64 * 2 * vpa + g0 - 1,
                        [[2 * vpa, 64], [vpa - 32, 2], [1, PAIR]])
            nc.sync.dma_start(up_dst, src)
            nc.sync.dma_start(hc16f[0:64, g0:g0 + PAIR], hff)
            nc.sync.dma_start(y_h[:, t * segtot + i0: t * segtot + i0 + PAIR],
                              hff)

        for t in range(t_steps):
            if t == 0:
                for (a, b2) in xpieces:
                    emit_x_piece(0, a, b2)
            pend_c = None    # (g, p0, p1) awaiting el-compute
            pend_d = []      # [(g, hf, hq)] awaiting dup DMAs
            for g in range(npair):
                res = emit_mm(t, g)
                if t + 1 < t_steps:
                    for k, (a, b2) in enumerate(xpieces):
                        if xpiece_after[k] == g:
                            emit_x_piece(t + 1, a, b2)
                if pend_c is not None:
                    gg, pp0, pp1 = pend_c
                    pend_d.append((gg,) + emit_elc(t, gg, pp0, pp1))
                pend_c = (g,) + res
                while pend_d and pend_d[0][0] <= g - 2:
                    gg, hf, hq = pend_d.pop(0)
                    emit_dups(t, gg, hf, hq)
            gg, pp0, pp1 = pend_c
            pend_d.append((gg,) + emit_elc(t, gg, pp0, pp1))
            for gg, hf, hq in pend_d:
                emit_dups(t, gg, hf, hq)

    nc.finalize()
    return nc


# ---------------------------------------------------------------------------
# host-side input prep

def prep_weights(Wc, b):
    Wc = np.asarray(Wc, np.float32)
    w8 = np.zeros((128, 12, 256), np.float32)
    s = 0
    for kd in (-1, 0, 1):
        for kh in (-1, 0, 1):
            w8[0:64, s, :] = Wc[:, 4:68, kd + 1, kh + 1, 0].T * WSCALE
            w8[64:128, s, :] = Wc[:, 4:68, kd + 1, kh + 1, 1].T * WSCALE
            s += 1
    for kd in (-1, 0, 1):
        w8[0:64, s, :] = Wc[:, 4:68, kd + 1, 0, 2].T * WSCALE
        w8[64:128, s, :] = Wc[:, 4:68, kd + 1, 1, 2].T * WSCALE
        s += 1
    ww8 = np.clip(w8, -240, 240).reshape(128, -1).astype(E4)

    w16 = np.zeros((128, 3, 256), np.float32)
    for ki, kd in enumerate((-1, 0, 1)):
        w16[0:64, ki, :] = Wc[:, 4:68, kd + 1, 2, 2].T * WSCALE
        for ka in range(3):
            for kb in range(3):
                j = 3 * ka + kb
                for c in range(C_IN):
                    w16[64 + 4 * j + c, ki, :] = Wc[:, c, kd + 1, ka, kb] * WSCALE
    ww16 = w16.reshape(128, -1).astype(BF)

    bias = np.zeros((128, 4), np.float32)
    b = np.asarray(b, np.float32)
    bias[:, 0] = b[0:128]
    bias[0:64, 1] = b[128:192]
    bias[0:64, 2] = b[192:256]
    return ww8, ww16, bias


def _vol_index(d_depth):
    d, r, c = np.meshgrid(np.arange(d_depth), np.arange(32), np.arange(32),
                          indexing="ij")
    return (d * PL + r * PLR + c).ravel()


def prep_peep(W_ci, W_cf, W_co, d_depth=D):
    _, pos0, nseg, _ = geom(d_depth)
    segtot = nseg * SEG
    idx = _vol_index(d_depth)
    pp = np.zeros((128, segtot), np.float32)
    pp[0:64, idx] = np.asarray(W_ci, np.float32).reshape(64, -1) * WSCALE
    pp[64:128, idx] = np.asarray(W_cf, np.float32).reshape(64, -1) * WSCALE
    ppom = np.zeros((128, segtot), np.float32)
    ppom[64:128, idx] = np.asarray(W_co, np.float32).reshape(64, -1) * WSCALE
    ppom[0:64, idx] = 1.0
    return (np.clip(pp, -240, 240).astype(E4),
            np.clip(ppom, -240, 240).astype(E4))


def prep_x(Xb, t_steps=T, d_depth=D):
    vp = (d_depth + 2) * PL
    xp = np.zeros((C_IN, t_steps, d_depth + 2, PLR, PLR), np.float32)
    xp[:, :, 1:1 + d_depth, 1:33, 1:33] = Xb
    return xp.reshape(C_IN, t_steps, vp).astype(BF)


def unpad_y(y, t_steps=T, d_depth=D):
    _, pos0, nseg, _ = geom(d_depth)
    segtot = nseg * SEG
    idx = _vol_index(d_depth)
    yv = np.asarray(y, np.float32).reshape(64, t_steps, segtot)
    return yv[:, :, idx].reshape(64, t_steps, d_depth, 32, 32)


_NC_CACHE = {}
_LAST_RESULTS = {}


def _get_nc():
    if "nc" not in _NC_CACHE:
        _NC_CACHE["nc"] = build_nc()
    return _NC_CACHE["nc"]


def kernel(X, Wc, b, W_ci, W_cf, W_co):
    import os
    from concourse.bass_utils import run_bass_kernel_spmd

    X = np.asarray(X, np.float32)
    ww8, ww16, bias = prep_weights(Wc, b)
    pp, ppom = prep_peep(W_ci, W_cf, W_co)
    in_maps = []
    for bi in range(B):
        in_maps.append({
            "xpad": prep_x(X[bi]),
            "ww8": ww8,
            "ww16": ww16,
            "bias": bias,
            "pp": pp,
            "ppom": ppom,
        })
    nc = _get_nc()
    trace = os.environ.get("TRACE_BASS", "0") == "1"
    res = run_bass_kernel_spmd(nc, in_maps, core_ids=list(range(B)), trace=trace)
    _LAST_RESULTS["br"] = res
    out = np.stack([
        unpad_y(res.results[bi]["y"]).reshape(C_OUT, T, D, HS, WS)
        for bi in range(B)
    ], axis=0)
    return out
